# revision 13
# baseline (speedup 1.0000x reference)
"""Trainium2 Bass kernel for CompressionSDF (4,128,128,128) -> (4,128,128,128).

Structure of the computation:
  stage: 1x1-conv stack over (B,C=128,H,W): 128->64->32->16 (lrelu, lrelu, none)
  then per-voxel MLP over a z-broadcast 17-channel field: 17->32->32->16->1
  (lrelu x3, sigmoid), where channel 16 is a z linspace coordinate.

Sharding: H axis split across 8 cores (16 rows each). Per core 8192 pixels,
1,048,576 voxels.

Per-core kernel layout: voxels are packed 4-per-column: partition dim holds
4 z-groups x 32 channels; columns are (z_lo, pixel). Layer 1 is computed by a
"selector" matmul whose stationary operand holds the per-pixel conv features
(so the z-broadcast never materializes in HBM); layers 2/3/4 use block-diagonal
weight matrices.

Schedule (v2): matmuls are emitted layer-major within each 64-pixel block so
the PE streams with one stationary per group (fewer LDWEIGHTS, sustained
p-state). PSUM tiles are [128,1024] (2 banks) so every evacuation instruction
covers two 512-col chunks, amortizing fixed per-instruction overhead. One
shared PSUM pool rotates p1 -> p2 -> p3 generations (the WAR dependency on the
pool buffer coincides with the dataflow dependency). Evacuation engine split:
ScalarE does h2/h3 Prelu(+true bias) and the batched sigmoid; DVE copies p1 to
SBUF (bf16) and GpSimd (no PSUM port) applies lrelu as (x*a) max x on SBUF.
L4 outputs of all 4 pair-chunks of a 2-block group land in one PSUM bank at
partition offsets 0/32/64/96 (stationary padded to 32 cols so every partition
is written) -> a single Sigmoid pass per 2 blocks. Stage conv biases for
layers 2/3 ride constant-1 input rows so those evacuations need no bias.
"""

import sys

sys.path.insert(0, "/opt/trn_rl_repo")

import numpy as np
from contextlib import ExitStack

import concourse.bass as bass
import concourse.tile as tile
from concourse import bacc, mybir
from concourse.bass_utils import run_bass_kernel_spmd

F32 = mybir.dt.float32
BF16 = mybir.dt.bfloat16
AF = mybir.ActivationFunctionType
ALU = mybir.AluOpType

N_CORES = 8
B, C, H, W, D = 4, 128, 128, 128, 128
HL = H // N_CORES            # h rows per core = 16
PIX = B * HL * W             # pixels per core = 8192
PB = 64                      # pixels per block
NBLK = PIX // PB             # 128 blocks
NGRP = NBLK // 2             # 64 groups of 2 blocks
NCH = 4                      # chunks per block, 8 z_lo each -> 32 z_lo
ALPHA = 0.01                 # LeakyReLU slope

_CACHE = {}


def _build_program(trace=False):
    nc = bacc.Bacc(
        "TRN2",
        target_bir_lowering=False,
        debug=False,
        enable_asserts=False,
        num_devices=N_CORES,
    )

    def din(name, shape, dt=F32):
        return nc.dram_tensor(name, list(shape), dt, kind="ExternalInput")

    x_d = din("x_sb", (C, PIX), BF16)
    w1_d = din("w1T", (128, 64), BF16)
    b1_d = din("b1c", (64, 1))
    w2_d = din("w2T", (65, 32), BF16)       # row 64 = sb2 (bias via const row)
    w3_d = din("w3T", (33, 16), BF16)       # row 32 = sb3
    gw_d = din("gw", (17, 128), BF16)
    selrow_d = din("selrow", (2, 128), BF16)
    selrhs_d = din("selrhs", (66, NCH * 512), BF16)
    l2_d = din("l2T", (128, 128), BF16)
    b2c4_d = din("b2c4", (128, 1))
    l3_d = din("l3T", (128, 128), BF16)
    b3c8_d = din("b3c8", (128, 1))
    l4_d = din("l4T32", (128, 32), BF16)    # cols 0..7 kron(I8,mw4.T), rest 0
    mb4_d = din("mb4c", (128, 1))
    out_d = nc.dram_tensor("out_sd", [D, PIX], F32, kind="ExternalOutput")

    with tile.TileContext(nc) as tc, ExitStack() as octx:
        cpool = octx.enter_context(tc.tile_pool(name="consts", bufs=1))

        def load(name, dram, shape, dt=F32):
            t = cpool.tile(list(shape), dt, name=name)
            nc.sync.dma_start(out=t[:], in_=dram[:])
            return t

        xt = load("xt", x_d, (C, PIX), BF16)
        w1s = load("w1s", w1_d, (128, 64), BF16)
        b1s = load("b1s", b1_d, (64, 1))
        w2s = load("w2s", w2_d, (65, 32), BF16)
        w3s = load("w3s", w3_d, (33, 16), BF16)
        gws = load("gws", gw_d, (17, 128), BF16)
        selrhss = load("selrhss", selrhs_d, (66, NCH * 512), BF16)
        l2s = load("l2s", l2_d, (128, 128), BF16)
        b2c4s = load("b2c4s", b2c4_d, (128, 1))
        l3s = load("l3s", l3_d, (128, 128), BF16)
        b3c8s = load("b3c8s", b3c8_d, (128, 1))
        l4s = load("l4s", l4_d, (128, 32), BF16)
        mb4s = load("mb4s", mb4_d, (128, 1))

        f1 = cpool.tile([65, PIX], BF16, name="f1")
        f2 = cpool.tile([33, PIX], BF16, name="f2")
        f3 = cpool.tile([17, PIX], BF16, name="f3")
        nc.vector.memset(f1[64:65, :], 1.0)   # const row feeds sb2 via matmul
        nc.vector.memset(f2[32:33, :], 1.0)   # const row feeds sb3
        # const row 16 feeds mb1 via gw; rows 0..15 overwritten by conv3
        # (memset whole tile: compute partition starts must be 32-aligned)
        nc.vector.memset(f3[:], 1.0)

        # 4 static selector-stationary tiles (A/B x double-buffer); aux rows
        # 64:66 (z-coordinate rows) are constant, written once.
        lhsTbs = []
        for k in range(4):
            t = cpool.tile([66, 128], BF16, name=f"lhsTb{k}")
            nc.sync.dma_start(out=t[64:66, :], in_=selrow_d[:])
            lhsTbs.append(t)

        # ---- stage: pointwise conv stack over pixels ----
        # layer-major so each conv's stationary stays resident; [.,1024]
        # PSUM tiles halve evacuation instruction count
        with tc.tile_pool(name="psA", bufs=2, space="PSUM") as psA:
            for t in range(PIX // 1024):
                s = bass.ts(t, 1024)
                pa = psA.tile([64, 1024], F32, name="pa")
                for h in range(2):
                    nc.tensor.matmul(pa[:, h * 512:(h + 1) * 512], w1s[:],
                                     xt[:, 1024 * t + 512 * h:
                                        1024 * t + 512 * (h + 1)],
                                     start=True, stop=True)
                nc.scalar.activation(f1[0:64, s], pa[:], AF.Prelu,
                                     bias=b1s[:], alpha=ALPHA)
        with tc.tile_pool(name="psB", bufs=2, space="PSUM") as psB:
            for t in range(PIX // 1024):
                s = bass.ts(t, 1024)
                pb = psB.tile([32, 1024], F32, name="pb")
                for h in range(2):
                    nc.tensor.matmul(pb[:, h * 512:(h + 1) * 512], w2s[:],
                                     f1[:, 1024 * t + 512 * h:
                                        1024 * t + 512 * (h + 1)],
                                     start=True, stop=True)
                # bias already in pb via const row -> Prelu without bias
                nc.scalar.activation(f2[0:32, s], pb[:], AF.Prelu, alpha=ALPHA)
        with tc.tile_pool(name="psC", bufs=2, space="PSUM") as psC:
            for t in range(PIX // 1024):
                s = bass.ts(t, 1024)
                pc = psC.tile([16, 1024], F32, name="pc")
                for h in range(2):
                    nc.tensor.matmul(pc[:, h * 512:(h + 1) * 512], w3s[:],
                                     f2[:, 1024 * t + 512 * h:
                                        1024 * t + 512 * (h + 1)],
                                     start=True, stop=True)
                # bias included; no activation -> plain copy on DVE
                nc.vector.tensor_copy(f3[0:16, s], pc[:])

        # ---- per-voxel MLP ----
        # z row index = 32*zg + t, t = 8*chunk + j
        osd = out_d[:].rearrange("(zg t) n -> zg t n", zg=4)

        with tc.tile_pool(name="psX", bufs=3, space="PSUM") as psX, \
             tc.tile_pool(name="psS", bufs=2, space="PSUM") as psS, \
             tc.tile_pool(name="c1pool", bufs=3) as c1pool, \
             tc.tile_pool(name="h1pool", bufs=3) as h1pool, \
             tc.tile_pool(name="h2pool", bufs=3) as h2pool, \
             tc.tile_pool(name="h3pool", bufs=2) as h3pool, \
             tc.tile_pool(name="sigp", bufs=2) as sigp:
            for g in range(NGRP):
                gs = bass.ts(g, 128)          # 128 pixels of this group
                # selector build: transpose+replicate both blocks' features
                pgp = psS.tile([128, 128], F32, name="pgp", tag="sg")
                nc.tensor.matmul(pgp[:], f3[:, gs], gws[:], start=True,
                                 stop=True)
                tA = lhsTbs[2 * (g % 2)]
                tB = lhsTbs[2 * (g % 2) + 1]
                nc.scalar.activation(tA[0:64, :], pgp[0:64, :], AF.Identity)
                nc.scalar.activation(tB[0:64, :], pgp[64:128, :], AF.Identity)

                psig = psS.tile([128, 512], F32, name="psig", tag="sg")
                for bi, lhsTb in enumerate((tA, tB)):
                    blk = 2 * g + bi
                    bs = bass.ts(blk, PB)
                    # L1: selector matmuls, one stationary for all 4 chunks;
                    # two 512-col halves per [128,1024] PSUM tile
                    p1s = [psX.tile([128, 1024], F32, name="p1", tag="x")
                           for _ in range(2)]
                    for c in range(NCH):
                        nc.tensor.matmul(
                            p1s[c // 2][:, (c % 2) * 512:(c % 2 + 1) * 512],
                            lhsTb[:], selrhss[:, bass.ts(c, 512)],
                            start=True, stop=True)
                    # h1 = lrelu(p1), no bias: V copies p1 to SBUF bf16,
                    # GpSimd makes the alpha-scaled copy (no PSUM port, and
                    # Pool tensor_tensor is rejected by the verifier), V does
                    # the final max at 2x bf16 rate.
                    h1s = []
                    for i in range(2):
                        c1 = c1pool.tile([128, 1024], BF16, name="c1")
                        nc.vector.tensor_copy(c1[:], p1s[i][:])
                        t1 = c1pool.tile([128, 1024], BF16, name="t1")
                        nc.gpsimd.tensor_scalar_mul(t1[:], c1[:], ALPHA)
                        h1 = h1pool.tile([128, 1024], BF16, name="h1")
                        nc.vector.tensor_tensor(h1[:], c1[:], t1[:],
                                                op=ALU.max)
                        h1s.append(h1)
                    # L2: block-diag matmuls, shared stationary
                    p2s = [psX.tile([128, 1024], F32, name="p2", tag="x")
                           for _ in range(2)]
                    for c in range(NCH):
                        nc.tensor.matmul(
                            p2s[c // 2][:, (c % 2) * 512:(c % 2 + 1) * 512],
                            l2s[:], h1s[c // 2][:, (c % 2) * 512:
                                                (c % 2 + 1) * 512],
                            start=True, stop=True)
                    # h2 = lrelu(p2 + b2): ScalarE Prelu with true bias
                    h2s = []
                    for i in range(2):
                        h2 = h2pool.tile([128, 1024], BF16, name="h2")
                        nc.scalar.activation(h2[:], p2s[i][:], AF.Prelu,
                                             bias=b2c4s[:], alpha=ALPHA)
                        h2s.append(h2)
                    # L3: pair u occupies cols u*512; chunk parity q picks the
                    # stationary half and output partition half. Order
                    # (c0,c2,c1,c3) keeps each stationary resident.
                    p3 = psX.tile([128, 1024], F32, name="p3", tag="x")
                    for c in (0, 2, 1, 3):
                        u, q = c // 2, c % 2
                        nc.tensor.matmul(
                            p3[q * 64:(q + 1) * 64, u * 512:(u + 1) * 512],
                            l3s[:, q * 64:(q + 1) * 64],
                            h2s[c // 2][:, (c % 2) * 512:(c % 2 + 1) * 512],
                            start=True, stop=True,
                            tile_position=(0, q * 64))
                    # h3 = lrelu(p3 + b3): ScalarE Prelu, both pairs at once
                    h3 = h3pool.tile([128, 1024], BF16, name="h3")
                    nc.scalar.activation(h3[:], p3[:], AF.Prelu,
                                         bias=b3c8s[:], alpha=ALPHA)
                    # L4 into shared psig bank at partition 32*P
                    for u in range(2):
                        P = 2 * bi + u
                        nc.tensor.matmul(psig[32 * P:32 * (P + 1), :],
                                         l4s[:], h3[:, u * 512:(u + 1) * 512],
                                         start=True, stop=True,
                                         tile_position=(0, 32 * P))
                # single sigmoid evacuation for the whole group (4 pairs)
                sig = sigp.tile([128, 512], F32, name="sig")
                nc.scalar.activation(sig[:], psig[:], AF.Sigmoid,
                                     bias=mb4s[:])
                # out DMA: P encodes (block-in-group bi, pair u); row
                # 32P+4qq+zg, col j*64+p -> z = 32zg+8(2u+qq)+j, pixel of blk
                for P in range(4):
                    bi, u = P // 2, P % 2
                    bs = bass.ts(2 * g + bi, PB)
                    for qq in range(2):
                        cc = 2 * u + qq
                        src = sig[32 * P + 4 * qq:32 * P + 4 * qq + 4, :]
                        src = src.rearrange("p (j w) -> p j w", j=8)
                        dst = osd[:, 8 * cc:8 * (cc + 1), bs]
                        nc.sync.dma_start(out=dst, in_=src)

    nc.compile()
    return nc


def _host_inputs(x, sw1, sb1, sw2, sb2, sw3, sb3,
                 mw1, mb1, mw2, mb2, mw3, mb3, mw4, mb4):
    import ml_dtypes
    f = np.float32
    bf = ml_dtypes.bfloat16
    zt = np.linspace(-1.0, 1.0, D, dtype=np.float64)
    c1 = mw1[:, 16].astype(np.float64)
    W1f = mw1[:, :16]

    gw = np.zeros((17, 128), f)
    gw[:16, :] = np.tile(W1f.T, (1, 4))
    gw[16, :] = np.tile(mb1, 4)

    A = zt[::32]                      # z-group base coordinate, shape (4,)
    Bv = zt[:32] - zt[0]              # z_lo offset, shape (32,)
    selrow = np.zeros((2, 128), f)
    selrow[0] = np.repeat(A, 32) * np.tile(c1, 4)
    selrow[1] = np.tile(c1, 4)

    selrhs = np.zeros((66, NCH * 512), f)
    eye_tiled = np.tile(np.eye(PB, dtype=f), (1, 8))   # [64, 512], col = j*64+p
    for c in range(NCH):
        s = slice(c * 512, (c + 1) * 512)
        selrhs[:PB, s] = eye_tiled
        selrhs[PB, s] = 1.0
        selrhs[PB + 1, s] = np.repeat(Bv[8 * c:8 * c + 8], PB)

    l4T32 = np.zeros((128, 32), f)
    l4T32[:, :8] = np.kron(np.eye(8, dtype=f), mw4.T)

    ins = {
        "w1T": np.ascontiguousarray(sw1.T).astype(bf),
        "b1c": sb1[:, None].astype(f),
        "w2T": np.vstack([sw2.T, sb2[None, :]]).astype(bf),
        "w3T": np.vstack([sw3.T, sb3[None, :]]).astype(bf),
        "gw": gw.astype(bf),
        "selrow": selrow.astype(bf),
        "selrhs": selrhs.astype(bf),
        "l2T": np.kron(np.eye(4, dtype=f), mw2.T).astype(bf),
        "b2c4": np.tile(mb2, 4)[:, None].astype(f),
        "l3T": np.concatenate([np.kron(np.eye(4, dtype=f), mw3.T)] * 2,
                              axis=1).astype(bf),
        "b3c8": np.tile(mb3, 8)[:, None].astype(f),
        "l4T32": l4T32.astype(bf),
        "mb4c": np.full((128, 1), mb4[0], f),
    }
    in_maps = []
    for k in range(N_CORES):
        xs = x[:, :, k * HL:(k + 1) * HL, :]
        xcore = np.ascontiguousarray(
            xs.transpose(1, 0, 2, 3).reshape(C, PIX)).astype(bf)
        in_maps.append({**ins, "x_sb": xcore})
    return in_maps


def run(trace=False, **inputs):
    if "nc" not in _CACHE:
        _CACHE["nc"] = _build_program()
    nc = _CACHE["nc"]
    in_maps = _host_inputs(**inputs)
    res = run_bass_kernel_spmd(nc, in_maps, list(range(N_CORES)), trace=trace)
    out = np.empty((B, D, H, W), np.float32)
    for k in range(N_CORES):
        o = res.results[k]["out_sd"].reshape(D, B, HL, W).transpose(1, 0, 2, 3)
        out[:, :, k * HL:(k + 1) * HL, :] = o
    return out, res


def kernel(**inputs):
    out, _ = run(trace=False, **inputs)
    return out


# revision 15
# speedup vs baseline: 4.1827x; 4.1827x over previous
"""Trainium2 Bass kernel for CompressionSDF (4,128,128,128) -> (4,128,128,128).

Structure of the computation:
  stage: 1x1-conv stack over (B,C=128,H,W): 128->64->32->16 (lrelu, lrelu, none)
  then per-voxel MLP over a z-broadcast 17-channel field: 17->32->32->16->1
  (lrelu x3, sigmoid), where channel 16 is a z linspace coordinate.

Sharding: H axis split across 8 cores (16 rows each). Per core 8192 pixels,
1,048,576 voxels.

Per-core kernel layout: voxels are packed 4-per-column: partition dim holds
4 z-groups x 32 channels; columns are (z_lo, pixel). Layer 1 is computed by a
"selector" matmul whose stationary operand holds the per-pixel conv features
(so the z-broadcast never materializes in HBM); layers 2/3/4 use block-diagonal
weight matrices.

Schedule (v2): matmuls are emitted layer-major within each 64-pixel block so
the PE streams with one stationary per group (fewer LDWEIGHTS, sustained
p-state). PSUM tiles are [128,1024] (2 banks) so every evacuation instruction
covers two 512-col chunks, amortizing fixed per-instruction overhead. One
shared PSUM pool rotates p1 -> p2 -> p3 generations (the WAR dependency on the
pool buffer coincides with the dataflow dependency). Evacuation engine split:
ScalarE does h2/h3 Prelu(+true bias) and the batched sigmoid; DVE copies p1 to
SBUF (bf16) and GpSimd (no PSUM port) applies lrelu as (x*a) max x on SBUF.
L4 outputs of all 4 pair-chunks of a 2-block group land in one PSUM bank at
partition offsets 0/32/64/96 (stationary padded to 32 cols so every partition
is written) -> a single Sigmoid pass per 2 blocks. Stage conv biases for
layers 2/3 ride constant-1 input rows so those evacuations need no bias.
"""

import sys

sys.path.insert(0, "/opt/trn_rl_repo")

import numpy as np
from contextlib import ExitStack

import concourse.bass as bass
import concourse.tile as tile
from concourse import bacc, mybir
from concourse.bass_utils import run_bass_kernel_spmd

F32 = mybir.dt.float32
BF16 = mybir.dt.bfloat16
AF = mybir.ActivationFunctionType
ALU = mybir.AluOpType

N_CORES = 8
B, C, H, W, D = 4, 128, 128, 128, 128
HL = H // N_CORES            # h rows per core = 16
PIX = B * HL * W             # pixels per core = 8192
PB = 64                      # pixels per block
NBLK = PIX // PB             # 128 blocks
NGRP = NBLK // 2             # 64 groups of 2 blocks
NCH = 4                      # chunks per block, 8 z_lo each -> 32 z_lo
ALPHA = 0.01                 # LeakyReLU slope

_CACHE = {}


def _build_program(trace=False):
    nc = bacc.Bacc(
        "TRN2",
        target_bir_lowering=False,
        debug=False,
        enable_asserts=False,
        num_devices=N_CORES,
    )

    def din(name, shape, dt=F32):
        return nc.dram_tensor(name, list(shape), dt, kind="ExternalInput")

    x_d = din("x_sb", (C, PIX), BF16)
    w1_d = din("w1T", (128, 64), BF16)
    b1_d = din("b1c", (64, 1))
    w2_d = din("w2T", (65, 32), BF16)       # row 64 = sb2 (bias via const row)
    w3_d = din("w3T", (33, 16), BF16)       # row 32 = sb3
    gw_d = din("gw", (17, 128), BF16)
    selrow_d = din("selrow", (2, 128), BF16)
    selrhs_d = din("selrhs", (66, NCH * 512), BF16)
    l2_d = din("l2T", (128, 128), BF16)
    b2c4_d = din("b2c4", (128, 1))
    l3_d = din("l3T", (128, 128), BF16)
    b3c8_d = din("b3c8", (128, 1))
    l4_d = din("l4T32", (128, 32), BF16)    # cols 0..7 kron(I8,mw4.T), rest 0
    mb4_d = din("mb4c", (128, 1))
    out_d = nc.dram_tensor("out_sd", [D, PIX], F32, kind="ExternalOutput")

    with tile.TileContext(nc) as tc, ExitStack() as octx:
        cpool = octx.enter_context(tc.tile_pool(name="consts", bufs=1))

        def load(name, dram, shape, dt=F32):
            t = cpool.tile(list(shape), dt, name=name)
            nc.sync.dma_start(out=t[:], in_=dram[:])
            return t

        xt = load("xt", x_d, (C, PIX), BF16)
        w1s = load("w1s", w1_d, (128, 64), BF16)
        b1s = load("b1s", b1_d, (64, 1))
        w2s = load("w2s", w2_d, (65, 32), BF16)
        w3s = load("w3s", w3_d, (33, 16), BF16)
        gws = load("gws", gw_d, (17, 128), BF16)
        selrhss = load("selrhss", selrhs_d, (66, NCH * 512), BF16)
        l2s = load("l2s", l2_d, (128, 128), BF16)
        b2c4s = load("b2c4s", b2c4_d, (128, 1))
        l3s = load("l3s", l3_d, (128, 128), BF16)
        b3c8s = load("b3c8s", b3c8_d, (128, 1))
        l4s = load("l4s", l4_d, (128, 32), BF16)
        mb4s = load("mb4s", mb4_d, (128, 1))

        f1 = cpool.tile([65, PIX], BF16, name="f1")
        f2 = cpool.tile([33, PIX], BF16, name="f2")
        f3 = cpool.tile([17, PIX], BF16, name="f3")
        nc.vector.memset(f1[64:65, :], 1.0)   # const row feeds sb2 via matmul
        nc.vector.memset(f2[32:33, :], 1.0)   # const row feeds sb3
        # const row 16 feeds mb1 via gw; rows 0..15 overwritten by conv3
        # (memset whole tile: compute partition starts must be 32-aligned)
        nc.vector.memset(f3[:], 1.0)

        # 4 static selector-stationary tiles (A/B x double-buffer); aux rows
        # 64:66 (z-coordinate rows) are constant, written once.
        lhsTbs = []
        for k in range(4):
            t = cpool.tile([66, 128], BF16, name=f"lhsTb{k}")
            nc.sync.dma_start(out=t[64:66, :], in_=selrow_d[:])
            lhsTbs.append(t)

        # ---- stage: pointwise conv stack over pixels ----
        # layer-major so each conv's stationary stays resident; [.,1024]
        # PSUM tiles halve evacuation instruction count
        with tc.tile_pool(name="psA", bufs=2, space="PSUM") as psA:
            for t in range(PIX // 1024):
                s = bass.ts(t, 1024)
                pa = psA.tile([64, 1024], F32, name="pa")
                for h in range(2):
                    nc.tensor.matmul(pa[:, h * 512:(h + 1) * 512], w1s[:],
                                     xt[:, 1024 * t + 512 * h:
                                        1024 * t + 512 * (h + 1)],
                                     start=True, stop=True)
                nc.scalar.activation(f1[0:64, s], pa[:], AF.Prelu,
                                     bias=b1s[:], alpha=ALPHA)
        with tc.tile_pool(name="psB", bufs=2, space="PSUM") as psB:
            for t in range(PIX // 1024):
                s = bass.ts(t, 1024)
                pb = psB.tile([32, 1024], F32, name="pb")
                for h in range(2):
                    nc.tensor.matmul(pb[:, h * 512:(h + 1) * 512], w2s[:],
                                     f1[:, 1024 * t + 512 * h:
                                        1024 * t + 512 * (h + 1)],
                                     start=True, stop=True)
                # bias already in pb via const row -> Prelu without bias
                nc.scalar.activation(f2[0:32, s], pb[:], AF.Prelu, alpha=ALPHA)
        with tc.tile_pool(name="psC", bufs=2, space="PSUM") as psC:
            for t in range(PIX // 1024):
                s = bass.ts(t, 1024)
                pc = psC.tile([16, 1024], F32, name="pc")
                for h in range(2):
                    nc.tensor.matmul(pc[:, h * 512:(h + 1) * 512], w3s[:],
                                     f2[:, 1024 * t + 512 * h:
                                        1024 * t + 512 * (h + 1)],
                                     start=True, stop=True)
                # bias included; no activation -> plain copy on DVE
                nc.vector.tensor_copy(f3[0:16, s], pc[:])

        # ---- per-voxel MLP ----
        # z row index = 32*zg + t, t = 8*chunk + j
        osd = out_d[:].rearrange("(zg t) n -> zg t n", zg=4)

        with tc.tile_pool(name="psX", bufs=3, space="PSUM") as psX, \
             tc.tile_pool(name="psS", bufs=2, space="PSUM") as psS, \
             tc.tile_pool(name="c1pool", bufs=3) as c1pool, \
             tc.tile_pool(name="h1pool", bufs=3) as h1pool, \
             tc.tile_pool(name="h2pool", bufs=3) as h2pool, \
             tc.tile_pool(name="h3pool", bufs=2) as h3pool, \
             tc.tile_pool(name="sigp", bufs=2) as sigp:
            for g in range(NGRP):
                gs = bass.ts(g, 128)          # 128 pixels of this group
                # selector build: transpose+replicate both blocks' features
                pgp = psS.tile([128, 128], F32, name="pgp", tag="sg")
                nc.tensor.matmul(pgp[:], f3[:, gs], gws[:], start=True,
                                 stop=True)
                tA = lhsTbs[2 * (g % 2)]
                tB = lhsTbs[2 * (g % 2) + 1]
                nc.vector.tensor_copy(tA[0:64, :], pgp[0:64, :])
                nc.vector.tensor_copy(tB[0:64, :], pgp[64:128, :])

                psig = psS.tile([128, 512], F32, name="psig", tag="sg")
                for bi, lhsTb in enumerate((tA, tB)):
                    blk = 2 * g + bi
                    bs = bass.ts(blk, PB)
                    # L1: selector matmuls, one stationary for all 4 chunks;
                    # two 512-col halves per [128,1024] PSUM tile
                    p1s = [psX.tile([128, 1024], F32, name="p1", tag="x")
                           for _ in range(2)]
                    for c in range(NCH):
                        nc.tensor.matmul(
                            p1s[c // 2][:, (c % 2) * 512:(c % 2 + 1) * 512],
                            lhsTb[:], selrhss[:, bass.ts(c, 512)],
                            start=True, stop=True)
                    # h1 = lrelu(p1), no bias. Half a: ScalarE Prelu. Half b:
                    # all-DVE 3-pass (PSUM copy, then scaled copy and max on
                    # SBUF bf16 where DVE runs its 2x/4x fast paths). GpSimd
                    # is avoided entirely: its elementwise ops run ~10x
                    # slower than modeled and contend for DVE's SBUF port.
                    h1s = []
                    h1 = h1pool.tile([128, 1024], BF16, name="h1")
                    nc.scalar.activation(h1[:], p1s[0][:], AF.Prelu,
                                         alpha=ALPHA)
                    h1s.append(h1)
                    c1 = c1pool.tile([128, 1024], BF16, name="c1")
                    nc.vector.tensor_copy(c1[:], p1s[1][:])
                    t1 = c1pool.tile([128, 1024], BF16, name="t1")
                    nc.vector.tensor_scalar_mul(t1[:], c1[:], ALPHA)
                    h1 = h1pool.tile([128, 1024], BF16, name="h1")
                    nc.vector.tensor_tensor(h1[:], c1[:], t1[:], op=ALU.max)
                    h1s.append(h1)
                    # L2: block-diag matmuls, shared stationary
                    p2s = [psX.tile([128, 1024], F32, name="p2", tag="x")
                           for _ in range(2)]
                    for c in range(NCH):
                        nc.tensor.matmul(
                            p2s[c // 2][:, (c % 2) * 512:(c % 2 + 1) * 512],
                            l2s[:], h1s[c // 2][:, (c % 2) * 512:
                                                (c % 2 + 1) * 512],
                            start=True, stop=True)
                    # h2 = lrelu(p2 + b2): ScalarE Prelu with true bias
                    h2s = []
                    for i in range(2):
                        h2 = h2pool.tile([128, 1024], BF16, name="h2")
                        nc.scalar.activation(h2[:], p2s[i][:], AF.Prelu,
                                             bias=b2c4s[:], alpha=ALPHA)
                        h2s.append(h2)
                    # L3: pair u occupies cols u*512; chunk parity q picks the
                    # stationary half and output partition half. Order
                    # (c0,c2,c1,c3) keeps each stationary resident.
                    p3 = psX.tile([128, 1024], F32, name="p3", tag="x")
                    for c in (0, 2, 1, 3):
                        u, q = c // 2, c % 2
                        nc.tensor.matmul(
                            p3[q * 64:(q + 1) * 64, u * 512:(u + 1) * 512],
                            l3s[:, q * 64:(q + 1) * 64],
                            h2s[c // 2][:, (c % 2) * 512:(c % 2 + 1) * 512],
                            start=True, stop=True,
                            tile_position=(0, q * 64))
                    # h3 = lrelu(p3 + b3): ScalarE Prelu, both pairs at once
                    h3 = h3pool.tile([128, 1024], BF16, name="h3")
                    nc.scalar.activation(h3[:], p3[:], AF.Prelu,
                                         bias=b3c8s[:], alpha=ALPHA)
                    # L4 into shared psig bank at partition 32*P
                    for u in range(2):
                        P = 2 * bi + u
                        nc.tensor.matmul(psig[32 * P:32 * (P + 1), :],
                                         l4s[:], h3[:, u * 512:(u + 1) * 512],
                                         start=True, stop=True,
                                         tile_position=(0, 32 * P))
                # single sigmoid evacuation for the whole group (4 pairs)
                sig = sigp.tile([128, 512], F32, name="sig")
                nc.scalar.activation(sig[:], psig[:], AF.Sigmoid,
                                     bias=mb4s[:])
                # out DMA: P encodes (block-in-group bi, pair u); row
                # 32P+4qq+zg, col j*64+p -> z = 32zg+8(2u+qq)+j, pixel of blk
                for P in range(4):
                    bi, u = P // 2, P % 2
                    bs = bass.ts(2 * g + bi, PB)
                    for qq in range(2):
                        cc = 2 * u + qq
                        src = sig[32 * P + 4 * qq:32 * P + 4 * qq + 4, :]
                        src = src.rearrange("p (j w) -> p j w", j=8)
                        dst = osd[:, 8 * cc:8 * (cc + 1), bs]
                        nc.sync.dma_start(out=dst, in_=src)

    nc.compile()
    return nc


def _host_inputs(x, sw1, sb1, sw2, sb2, sw3, sb3,
                 mw1, mb1, mw2, mb2, mw3, mb3, mw4, mb4):
    import ml_dtypes
    f = np.float32
    bf = ml_dtypes.bfloat16
    zt = np.linspace(-1.0, 1.0, D, dtype=np.float64)
    c1 = mw1[:, 16].astype(np.float64)
    W1f = mw1[:, :16]

    gw = np.zeros((17, 128), f)
    gw[:16, :] = np.tile(W1f.T, (1, 4))
    gw[16, :] = np.tile(mb1, 4)

    A = zt[::32]                      # z-group base coordinate, shape (4,)
    Bv = zt[:32] - zt[0]              # z_lo offset, shape (32,)
    selrow = np.zeros((2, 128), f)
    selrow[0] = np.repeat(A, 32) * np.tile(c1, 4)
    selrow[1] = np.tile(c1, 4)

    selrhs = np.zeros((66, NCH * 512), f)
    eye_tiled = np.tile(np.eye(PB, dtype=f), (1, 8))   # [64, 512], col = j*64+p
    for c in range(NCH):
        s = slice(c * 512, (c + 1) * 512)
        selrhs[:PB, s] = eye_tiled
        selrhs[PB, s] = 1.0
        selrhs[PB + 1, s] = np.repeat(Bv[8 * c:8 * c + 8], PB)

    l4T32 = np.zeros((128, 32), f)
    l4T32[:, :8] = np.kron(np.eye(8, dtype=f), mw4.T)

    ins = {
        "w1T": np.ascontiguousarray(sw1.T).astype(bf),
        "b1c": sb1[:, None].astype(f),
        "w2T": np.vstack([sw2.T, sb2[None, :]]).astype(bf),
        "w3T": np.vstack([sw3.T, sb3[None, :]]).astype(bf),
        "gw": gw.astype(bf),
        "selrow": selrow.astype(bf),
        "selrhs": selrhs.astype(bf),
        "l2T": np.kron(np.eye(4, dtype=f), mw2.T).astype(bf),
        "b2c4": np.tile(mb2, 4)[:, None].astype(f),
        "l3T": np.concatenate([np.kron(np.eye(4, dtype=f), mw3.T)] * 2,
                              axis=1).astype(bf),
        "b3c8": np.tile(mb3, 8)[:, None].astype(f),
        "l4T32": l4T32.astype(bf),
        "mb4c": np.full((128, 1), mb4[0], f),
    }
    in_maps = []
    for k in range(N_CORES):
        xs = x[:, :, k * HL:(k + 1) * HL, :]
        xcore = np.ascontiguousarray(
            xs.transpose(1, 0, 2, 3).reshape(C, PIX)).astype(bf)
        in_maps.append({**ins, "x_sb": xcore})
    return in_maps


def run(trace=False, **inputs):
    if "nc" not in _CACHE:
        _CACHE["nc"] = _build_program()
    nc = _CACHE["nc"]
    in_maps = _host_inputs(**inputs)
    res = run_bass_kernel_spmd(nc, in_maps, list(range(N_CORES)), trace=trace)
    out = np.empty((B, D, H, W), np.float32)
    for k in range(N_CORES):
        o = res.results[k]["out_sd"].reshape(D, B, HL, W).transpose(1, 0, 2, 3)
        out[:, :, k * HL:(k + 1) * HL, :] = o
    return out, res


def kernel(**inputs):
    out, _ = run(trace=False, **inputs)
    return out
